# revision 37
# baseline (speedup 1.0000x reference)
"""Trainium2 Bass kernel: top-2 MoE routing (E=16, D=H=2048), 8 NeuronCores.

Strategy (memory-regime: only the 2 selected experts' weights are read):
  * Every core redundantly computes the gating on-device from one packed
    bf16 tile (x, Wg.T, bg): logits via 17 accumulating PE matmuls, then
    top-2 values+indices in two DVE ops (max8 / max_index).
  * Weights are sharded across cores *within* each expert (core c owns
    rows [c*256,(c+1)*256) of W1 and the matching contraction slice of
    W2), pre-transposed AND pre-cast to bf16 on the host so each
    expert-slice is one contiguous 1MB dynamic-offset DMA (b1 rides in
    the W1 DMA's tail columns).  bf16 halves the HBM traffic; rel err
    stays ~5e-3 (tolerance 2e-2).
  * Layer 1: weights-stationary matmuls -> h [128,2] PSUM; tanh(+b1) on
    Act; gate-weighting folded into h (h *= tkg_k) so both experts and
    the tkg-weighted b2 rows accumulate into ONE [128,16] PSUM tile.
  * Output element o = p*16 + c lives at PSUM [p, c] (host permutes W2's
    output dim), so the final store is a single cheap 128-descriptor DMA
    after one PSUM->SBUF copy.  Host sums the 8 per-core partials.
  * The second expert's W2 streams last, split (1MB | 14 cols | 2 cols)
    so only two matmuls + the copy + store hang off the final bytes.
"""

import numpy as np

try:  # make concourse importable in bare environments
    import concourse.bacc  # noqa: F401
except ImportError:  # pragma: no cover
    import sys

    sys.path.insert(0, "/opt/trn_rl_repo")

E, D, H = 16, 2048, 2048
NCORES = 8
P = 128
RS = H // NCORES  # 256 rows of each expert held per core
NCH = RS // P  # 2 row-chunks per core
DC = D // P  # 16 contraction chunks for layer 1
OC = H // P  # 16 output chunks for layer 2
GF = DC + DC * E + E + 1  # packed gating tile cols: x | Wg.T | bg | one

_BUILT = None


def _build():
    """Build + compile the Bass program once. Returns (nc, input_names)."""
    global _BUILT
    if _BUILT is not None:
        return _BUILT

    import concourse.bacc as bacc
    import concourse.bass as bass
    import concourse.tile as tile
    from concourse import mybir

    f32 = mybir.dt.float32
    bf16 = mybir.dt.bfloat16
    u32 = mybir.dt.uint32
    AX = mybir.AxisListType.X
    OP = mybir.AluOpType

    nc = bacc.Bacc(
        "TRN2",
        target_bir_lowering=False,
        debug=False,
        num_devices=1,
        dynamic_dma_scratch_size=2048,
    )

    # ----- I/O ------------------------------------------------------------
    gpack_d = nc.dram_tensor("gpack", [P, GF], bf16, kind="ExternalInput")
    w1e_d = nc.dram_tensor(
        "w1e", [E, P, DC * P * NCH + NCH], bf16, kind="ExternalInput"
    )
    w2e_d = nc.dram_tensor(
        "w2e", [E, P, NCH * P * OC], bf16, kind="ExternalInput"
    )
    b2p_d = nc.dram_tensor("b2p", [E, H], f32, kind="ExternalInput")
    out_d = nc.dram_tensor("out", [1, H], f32, kind="ExternalOutput")

    in_names = ["gpack", "w1e", "w2e", "b2p"]
    W1F = DC * P * NCH + NCH  # 4098
    W2F = NCH * P * OC  # 4096

    with tile.TileContext(nc) as tc:
        with (
            tc.tile_pool(name="sb", bufs=1) as sb,
            tc.tile_pool(name="ps", bufs=1, space="PSUM") as ps,
        ):
            # ----- packed gating load (one fast DMA) -----------------------
            gp = sb.tile([P, GF], bf16, tag="gp")
            nc.sync.dma_start(gp[:], gpack_d.ap())

            # ----- gating on PE: logits = Wg @ x + bg ----------------------
            lg_ps = ps.tile([1, E], f32, tag="lg_ps")
            nc.tensor.matmul(  # bg via K=1 matmul with the packed 1.0
                out=lg_ps[:],
                lhsT=gp[0:1, GF - 1 : GF],
                rhs=gp[0:1, DC + DC * E : DC + DC * E + E],
                start=True,
                stop=False,
            )
            for dc in range(DC):
                nc.tensor.matmul(
                    out=lg_ps[:],
                    lhsT=gp[:, dc : dc + 1],
                    rhs=gp[:, DC + dc * E : DC + (dc + 1) * E],
                    start=False,
                    stop=(dc == DC - 1),
                )

            # ----- top-2 values + indices in two DVE ops (PSUM-direct) -----
            mx = sb.tile([1, 8], f32, tag="mx")
            mi = sb.tile([1, 8], u32, tag="mi")
            nc.vector.max(mx[:], lg_ps[:])
            nc.vector.max_index(mi[:], mx[:], lg_ps[:])

            # ----- expert indices -> registers, weight DMAs ----------------
            sv = []  # SP-register index per expert slot (weights)
            pv = []  # Pool-register index per expert slot (b2 rows)
            w1t = [
                sb.tile([P, W1F], bf16, tag=f"w1t{k}", name=f"w1t{k}")
                for k in range(2)
            ]
            w2t = [
                sb.tile([P, W2F], bf16, tag=f"w2t{k}", name=f"w2t{k}")
                for k in range(2)
            ]
            for k in range(2):
                reg = nc.sync.alloc_register(f"idx_sp{k}")
                nc.sync.reg_load(reg, mi[:, k : k + 1])
                sv.append(nc.snap(reg, donate=True, min_val=0, max_val=E - 1))
                nc.sync.dma_start(
                    w1t[k][:],
                    w1e_d.ap()[bass.ds(sv[k], 1), :, :].rearrange(
                        "e p f -> p (e f)"
                    ),
                )
            # k=1's W2 is split by contraction half so the first half's
            # matmuls overlap the second half's stream (shorter tail)
            nc.sync.dma_start(
                w2t[0][:],
                w2e_d.ap()[bass.ds(sv[0], 1), :, :].rearrange(
                    "e p f -> p (e f)"
                ),
            )
            for lo, hi in [(0, H), (H, 2 * H - 2 * P), (2 * H - 2 * P, 2 * H)]:
                nc.sync.dma_start(
                    w2t[1][:, lo:hi],
                    w2e_d.ap()[bass.ds(sv[1], 1), :, lo:hi].rearrange(
                        "e p f -> p (e f)"
                    ),
                )

            # ----- normalized top-2 gates ---------------------------------
            #   e2 = exp(m2-m1); S = sum(exp(logits-m1))
            #   tkg0 = 1/(1 + e2 + 1e-6*S);  tkg1 = e2 * tkg0
            negm1 = sb.tile([1, 1], f32, tag="negm1")
            nc.vector.tensor_scalar_mul(negm1[:], mx[:, 0:1], -1.0)
            e2x = sb.tile([1, 1], f32, tag="e2x")
            nc.scalar.activation(
                e2x[:], mx[:, 1:2], mybir.ActivationFunctionType.Exp,
                bias=negm1[:],
            )
            esb = sb.tile([1, E], f32, tag="esb")
            nc.scalar.activation(
                esb[:], lg_ps[:], mybir.ActivationFunctionType.Exp,
                bias=negm1[:],
            )
            # b2 rows (pre-permuted, /NCORES) on the Pool engine
            b2r = [
                sb.tile([1, H], f32, tag=f"b2r{k}", name=f"b2r{k}")
                for k in range(2)
            ]
            for k in range(2):
                preg = nc.gpsimd.alloc_register(f"idx_pool{k}")
                nc.gpsimd.reg_load(preg, mi[:, k : k + 1])
                pv.append(nc.snap(preg, donate=True, min_val=0, max_val=E - 1))
                nc.gpsimd.dma_start(
                    b2r[k][:], b2p_d.ap()[bass.ds(pv[k], 1), :]
                )

            ssum = sb.tile([1, 1], f32, tag="ssum")
            nc.vector.tensor_reduce(ssum[:], esb[:], axis=AX, op=OP.add)
            den = sb.tile([1, 1], f32, tag="den")
            nc.vector.tensor_scalar(den[:], ssum[:], 1e-6, 1.0, OP.mult, OP.add)
            nc.vector.tensor_add(den[:], den[:], e2x[:])
            tkg = [
                sb.tile([1, 1], f32, tag=f"tkg{k}", name=f"tkg{k}")
                for k in range(2)
            ]
            nc.vector.reciprocal(tkg[0][:], den[:])
            nc.vector.tensor_mul(tkg[1][:], e2x[:], tkg[0][:])
            tkg_rep = [
                sb.tile([P, 1], f32, tag=f"tkgr{k}", name=f"tkgr{k}")
                for k in range(2)
            ]
            for k in range(2):
                nc.gpsimd.partition_broadcast(tkg_rep[k][:], tkg[k][:])

            # ----- layer 1: h = tanh(W1[e] @ x + b1[e]), then h *= tkg_k ---
            h_ps = [
                ps.tile([P, NCH], f32, tag=f"hps{k}", name=f"hps{k}")
                for k in range(2)
            ]
            hs = [
                sb.tile([P, NCH], bf16, tag=f"hs{k}", name=f"hs{k}")
                for k in range(2)
            ]
            for k in range(2):
                for rc in range(NCH):
                    for dc in range(DC):
                        nc.tensor.matmul(
                            out=h_ps[k][:, rc : rc + 1],
                            lhsT=w1t[k][
                                :, dc * RS + rc * P : dc * RS + (rc + 1) * P
                            ],
                            rhs=gp[:, dc : dc + 1],
                            start=(dc == 0),
                            stop=(dc == DC - 1),
                        )
                    nc.scalar.activation(
                        hs[k][:, rc : rc + 1],
                        h_ps[k][:, rc : rc + 1],
                        mybir.ActivationFunctionType.Tanh,
                        bias=w1t[k][:, DC * RS + rc : DC * RS + rc + 1],
                    )
                nc.vector.tensor_scalar(
                    hs[k][:], hs[k][:], tkg_rep[k][:], None, OP.mult
                )

            # ----- layer 2 + b2: everything accumulates into one PSUM tile -
            # eo_ps[p, c] = output element o = p*16 + c (host permuted W2/b2)
            eo_ps = ps.tile([P, OC], f32, tag="eo_ps")
            for k in range(2):  # + tkg_k * b2[e_k]/NCORES
                for c in range(OC):
                    nc.tensor.matmul(
                        out=eo_ps[:, c : c + 1],
                        lhsT=b2r[k][:, c * P : (c + 1) * P],
                        rhs=tkg[k][:],
                        start=(k == 0),
                        stop=False,
                    )
            for k in range(2):
                for ic in range(NCH):
                    for c in range(OC):
                        nc.tensor.matmul(
                            out=eo_ps[:, c : c + 1],
                            lhsT=w2t[k][
                                :, ic * H + c * P : ic * H + (c + 1) * P
                            ],
                            rhs=hs[k][:, ic : ic + 1],
                            start=False,
                            stop=(k == 1 and ic == NCH - 1),
                        )

            # ----- store: PSUM -> SBUF -> one contiguous-per-partition DMA -
            eo = sb.tile([P, OC], f32, tag="eo")
            nc.vector.tensor_copy(eo[:], eo_ps[:])
            nc.sync.dma_start(
                out_d.ap().rearrange("a (p c) -> p (a c)", p=P), eo[:]
            )

    nc.compile()
    _BUILT = (nc, in_names)
    return _BUILT


def make_in_maps(x, Wg, bg, W1, b1, W2, b2):
    """Host-side sharding: per-core input dicts (bf16 weight repack)."""
    import ml_dtypes

    bf16 = ml_dtypes.bfloat16
    x = np.asarray(x, np.float32).reshape(D)
    Wg = np.asarray(Wg, np.float32)
    bg = np.asarray(bg, np.float32).reshape(E)
    # cast once up front: halves the bytes every per-core transpose moves
    # (cast is elementwise, so cast-then-transpose == transpose-then-cast)
    W1 = np.asarray(W1, np.float32).astype(bf16)
    b1 = np.asarray(b1, np.float32).astype(bf16)
    W2 = np.asarray(W2, np.float32).astype(bf16)
    b2 = np.asarray(b2, np.float32)

    gpack = np.zeros((P, GF), np.float32)
    gpack[:, 0:DC] = x.reshape(DC, P).T  # x[dc*128+p] at [p, dc]
    # wgt[p, dc*16+e] = Wg[e, dc*128+p]
    gpack[:, DC : DC + DC * E] = (
        Wg.T.reshape(DC, P, E).transpose(1, 0, 2).reshape(P, DC * E)
    )
    gpack[0, DC + DC * E : DC + DC * E + E] = bg
    gpack[0, GF - 1] = 1.0
    gpack = np.ascontiguousarray(gpack.astype(bf16))

    # b2p[e, c*128+mm] = b2[e, mm*16+c] / NCORES
    b2p = np.ascontiguousarray(
        (b2 / NCORES).reshape(E, P, OC).transpose(0, 2, 1).reshape(E, H)
    )

    in_maps = []
    for c in range(NCORES):
        rs = slice(c * RS, (c + 1) * RS)
        # w1e[e, p, dc*256 + rc*128 + rr] = W1[e, rs0+rc*128+rr, dc*128+p]
        a = W1[:, rs, :].transpose(0, 2, 1)  # [E, d, r]
        a = a.reshape(E, DC, P, NCH, P).transpose(0, 2, 1, 3, 4)
        w1flat = a.reshape(E, P, DC * RS)
        # b1 tail cols: b1t[p, rc] = b1[e, rs0 + rc*128 + p]
        b1s = b1[:, rs].reshape(E, NCH, P).transpose(0, 2, 1)
        w1e = np.ascontiguousarray(np.concatenate([w1flat, b1s], axis=2))
        # w2e[e, p, ic*2048 + cc*128 + mm] = W2[e, mm*16+cc, ic*128+p]
        b_ = W2[:, :, rs].transpose(0, 2, 1)  # [E, i, o]
        b_ = b_.reshape(E, NCH, P, P, OC).transpose(0, 2, 1, 4, 3)
        w2e = np.ascontiguousarray(b_.reshape(E, P, NCH * P * OC))
        in_maps.append(
            {"gpack": gpack, "w1e": w1e, "w2e": w2e, "b2p": b2p}
        )
    return in_maps


def kernel(x, Wg, bg, W1, b1, W2, b2, train=0, **_unused):
    from concourse import bass_utils

    nc, _ = _build()
    in_maps = make_in_maps(x, Wg, bg, W1, b1, W2, b2)
    res = bass_utils.run_bass_kernel_spmd(
        nc, in_maps, core_ids=list(range(NCORES))
    )
    outs = [
        np.asarray(res.results[c]["out"], np.float32).reshape(H)
        for c in range(NCORES)
    ]
    # each core holds the gate-weighted partial sum over its contraction
    # shard; unshard by summing the partials
    return np.sum(outs, axis=0, dtype=np.float32)
